# revision 1
# baseline (speedup 1.0000x reference)
"""Trainium2 Bass kernel v2 for nn_Based_40630390620259 (sparse_attention).

Restructured from the v1 kernel around three ideas:

1. Transposed accumulation: qkv/window accumulators are computed as
   out[s(128 partitions), 65] = sum_t strip[t, s] @ v4[t, 65] so the PE
   streams 65 cols per (s-chunk, t-chunk) instead of 128 (the old [65, s]
   orientation wasted half the array's output partitions). Division by the
   denominator becomes a per-partition tensor_scalar (no partition
   broadcast), and the causal cumulative-v term is folded into the same
   PSUM groups via constant tril/selector matmuls (no serial CUM chain).

2. fp8 DoubleRow (0.5 cycles/row) for the windowed attention scores:
   q/k are quantized to e4m3 (x8 scale) and repacked [32, 2, S] per head
   (head-dim halves as DR subtiles); exp scale absorbs the 1/64. Optional
   same trick for the linear-attention scores (x16 scale, 8+8 subtiles,
   Square(in/256 + 1) folds the feature-map constant, no ones rows).

3. Retirement pipeline: finished acc banks are copied to SBUF, then
   reciprocal/divide/DMA-transpose/output-projection run as filler work
   interleaved into the next block's instruction stream.

Sharding: tensor-parallel over heads, 2 heads per core, 8 cores; each core
produces a partial [S, D] bf16 output (host sums in f32).
"""

import numpy as np
import ml_dtypes

S = 2048
D = 1024
H = 16
FD = 16
HD = 64
W = 256
NCORES = 8
NT = 16          # 128-row t/s chunks
SB = 512         # s block width (4 chunks)
NJ = 4

BF = ml_dtypes.bfloat16
E4 = ml_dtypes.float8_e4m3fn

LIN_DR = True    # fp8 DoubleRow for linear-attention scores

_CACHE = {}


def _build_nc(lin_dr=LIN_DR, dbg=False):
    import concourse.bass as bass
    import concourse.mybir as mybir
    import concourse.tile as tile
    from concourse import bacc
    from concourse.bass import ts

    f32 = mybir.dt.float32
    bf16 = mybir.dt.bfloat16
    fp8 = mybir.dt.float8e4
    MULT = mybir.AluOpType.mult
    ADD = mybir.AluOpType.add
    DR = mybir.MatmulPerfMode.DoubleRow
    Exp = mybir.ActivationFunctionType.Exp
    Square = mybir.ActivationFunctionType.Square
    Copy = mybir.ActivationFunctionType.Copy

    nc = bacc.Bacc("TRN2", target_bir_lowering=False)

    ht_d = nc.dram_tensor("ht", [D, S], bf16, kind="ExternalInput")
    # wqk cols: [winq a_lo32 a_hi32 b_lo32 b_hi32 | wink same | lin 64:
    #   qa16 ka16 qb16 kb16 (rows lo8 hi8 within each 16)]
    wqk_d = nc.dram_tensor("wqk", [D, 320], bf16, kind="ExternalInput")
    wv_d = nc.dram_tensor("wv", [D, 256], bf16, kind="ExternalInput")
    wo_d = nc.dram_tensor("wo", [256, D], bf16, kind="ExternalInput")
    msk_d = nc.dram_tensor("msk", [128, 384], bf16, kind="ExternalInput")
    tril_d = nc.dram_tensor("tril", [128, 128], bf16, kind="ExternalInput")
    bsel_d = nc.dram_tensor("bsel", [16, NT * 128], bf16, kind="ExternalInput")
    oneh_d = nc.dram_tensor("oneh", [128, NT * 16], bf16, kind="ExternalInput")
    orow_d = nc.dram_tensor("orow", [1, S], bf16, kind="ExternalInput")
    out_d = nc.dram_tensor("out", [S, D], bf16, kind="ExternalOutput")
    if dbg:
        dU_d = nc.dram_tensor("dU", [128, NJ * 1040], mybir.dt.float32,
                              kind="ExternalOutput")
        dscl_d = nc.dram_tensor("dscl", [128, NT * 256], bf16,
                                kind="ExternalOutput")
        dcs_d = nc.dram_tensor("dcs", [16, 130], bf16, kind="ExternalOutput")

    # DMA queue round-robin over the three HWDGE engines
    qrr = {"i": 0}

    def dma(out, in_, q=None):
        eng = (nc.sync, nc.scalar, nc.gpsimd)[
            q if q is not None else qrr["i"] % 2
        ]
        if q is None:
            qrr["i"] += 1
        eng.dma_start(out=out, in_=in_)

    def dma_t(out, in_):
        nc.sync.dma_start_transpose(out=out, in_=in_)

    with tile.TileContext(nc) as tc:
        with (
            tc.tile_pool(name="sb", bufs=1) as sb,
            tc.tile_pool(name="stp", bufs=10) as stp,
            tc.tile_pool(name="mcp", bufs=2) as mcp,
            tc.tile_pool(name="stg", bufs=4) as stg,
            tc.tile_pool(name="psA", bufs=1, space="PSUM") as psA,
            tc.tile_pool(name="psB", bufs=1, space="PSUM") as psB,
        ):
            # ---------------- persistent SBUF ----------------
            ht_sb = sb.tile([128, 8, S], bf16, name="ht_sb")
            wqk_sb = sb.tile([128, 8, 320], bf16, name="wqk_sb")
            wv_sb = sb.tile([128, 8, 256], bf16, name="wv_sb")
            wo_sb = sb.tile([128, 2, D], bf16, name="wo_sb")
            msk_sb = sb.tile([128, 384], bf16, name="msk_sb")
            tril_sb = sb.tile([128, 128], bf16, name="tril_sb")
            bsel_sb = sb.tile([16, NT * 128], bf16, name="bsel_sb")
            oneh_sb = sb.tile([128, NT * 16], bf16, name="oneh_sb")

            qw8 = sb.tile([64, 2, S], fp8, name="qw8")
            kw8 = sb.tile([64, 2, S], fp8, name="kw8")
            qws = sb.tile([128, S], fp8, name="qws")
            kws = sb.tile([128, S], fp8, name="kws")
            if lin_dr:
                # lin q/k: [8, 2, S] per (tensor, head); head a parts 0:8,
                # head b parts 32:40; axis1 = contraction subtile (lo/hi 8)
                qkg8 = sb.tile([64, 2, 2, S], fp8, name="qkg8")
                qks = sb.tile([64, S], fp8, name="qks")
            else:
                qkg_sb = sb.tile([128, 2, S], bf16, name="qkg_sb")
                qkb = sb.tile([64, S], bf16, name="qkb")

            v4_sb = sb.tile([128, NT, 260], bf16, name="v4_sb")
            csum_sb = sb.tile([16, 130], bf16, name="csum_sb")
            U_sb = sb.tile([128, NJ, 1040], f32, name="U_sb")
            rec_sb = sb.tile([128, NJ, 16], f32, name="rec_sb")
            scl_sb = sb.tile([128, NT, 256], bf16, name="scl_sb")
            sclT = sb.tile([128, 2, NT, 128], bf16, name="sclT")

            # ---------------- PSUM ----------------
            # psA: 2 x [128,1024] score strips (4 banks)
            # psB: 3 acc banks + po bank (manual layout)
            accP = [psB.tile([128, 512], f32, name=f"accP{i}") for i in range(3)]
            poP = psB.tile([128, 512], f32, name="poP")

            def acc_slot(G):
                b, o = (G // 7, G % 7) if G < 14 else (2, G - 14)
                return accP[b], o * 65

            # one start=True per (bank, j): the first matmul emitted to a
            # bank marks the whole 2KB zero-region pending; every slot's
            # first subsequent write then auto-zeroes (overwrite) and later
            # writes accumulate. Multiple starts per bank would poison other
            # slots' partial sums (pending-zero is bank-wide).
            opened = set()

            def bank_mm(G, lhsT, rhs, stop):
                bank, off = acc_slot(G)
                bid = 0 if G < 7 else (1 if G < 14 else 2)
                st = bid not in opened
                opened.add(bid)
                nc.tensor.matmul(
                    bank[:, off : off + 65],
                    lhsT=lhsT,
                    rhs=rhs,
                    start=st,
                    stop=stop,
                    skip_group_check=True,
                )

            # ---------------- input loads (few big DMAs) ----------------
            htr = ht_d.rearrange("(k p) s -> p k s", p=128)
            wqr = wqk_d.rearrange("(k p) x -> p k x", p=128)
            dma(wqk_sb[:, 0:4, :], wqr[:, 0:4, :], q=0)
            dma(wqk_sb[:, 4:8, :], wqr[:, 4:8, :], q=1)
            for q4 in range(4):
                dma(ht_sb[:, :, ts(q4, 512)], htr[:, :, ts(q4, 512)],
                    q=(0, 1, 0, 1)[q4])
            dma(wv_sb[:, :, :], wv_d.rearrange("(k p) x -> p k x", p=128), q=1)
            dma(wo_sb[:, :, :], wo_d.rearrange("(k p) x -> p k x", p=128), q=2)
            dma(msk_sb[:, :], msk_d[:, :], q=2)
            dma(tril_sb[:, :], tril_d[:, :], q=2)
            dma(bsel_sb[:, :], bsel_d[:, :], q=2)
            dma(oneh_sb[:, :], oneh_d[:, :], q=2)

            v4r = v4_sb.rearrange("p s (g x) -> p s g x", x=65)
            nc.gpsimd.memset(v4r[:, :, 0:2, 64], 0.5)
            nc.gpsimd.memset(v4r[:, :, 2:4, 64], 1.0)
            # preload the Exp activation table during phase 1 so the
            # 1.3us table load is off the first win strip's critical path
            nc.scalar.activation(
                rec_sb[0:1, 0, 0:1], wqk_sb[0:1, 0, 0:1], Exp, scale=1.0
            )

            # ---------------- phase 1a/1b ----------------

            def proj_blk(blk, jp):
                c0 = blk * 128
                wdt = 64 if blk == 2 else 128
                ppa = psA.tile([128, SB], f32, name="ppa", tag="mpa")
                ppb = psA.tile([128, SB], f32, name="ppb", tag="mpb")
                pph = (ppa, ppb)
                for jh in range(2):
                    for k in range(8):
                        nc.tensor.matmul(
                            pph[jh][0:wdt, :],
                            lhsT=wqk_sb[:, k, c0 : c0 + wdt],
                            rhs=ht_sb[:, k, ts(2 * jp + jh, SB)],
                            start=(k == 0),
                            stop=(k == 7),
                        )
                js2 = ts(jp, 1024)
                jsh = [ts(2 * jp, SB), ts(2 * jp + 1, SB)]
                if blk == 0:
                    for jh in range(2):
                        nc.scalar.activation(
                            qws[:, jsh[jh]], pph[jh][:, :], Copy, scale=8.0
                        )
                    for hh in range(2):
                        for sub in range(2):
                            dma(
                                qw8[32 * hh : 32 * hh + 32, sub, js2],
                                qws[64 * hh + 32 * sub : 64 * hh + 32 * sub + 32, js2],
                                q=2,
                            )
                elif blk == 1:
                    for jh in range(2):
                        nc.scalar.activation(
                            kws[:, jsh[jh]], pph[jh][:, :], Copy, scale=8.0
                        )
                    for hh in range(2):
                        for sub in range(2):
                            dma(
                                kw8[32 * hh : 32 * hh + 32, sub, js2],
                                kws[64 * hh + 32 * sub : 64 * hh + 32 * sub + 32, js2],
                                q=2,
                            )
                elif lin_dr:
                    for jh in range(2):
                        nc.scalar.activation(
                            qks[0:64, jsh[jh]], pph[jh][0:64, :], Copy,
                            scale=16.0,
                        )
                    for hh in range(2):
                        for qk in range(2):
                            for sub in range(2):
                                p0 = 32 * hh + 16 * qk + 8 * sub
                                dma(
                                    qkg8[32 * hh : 32 * hh + 8, qk, sub, js2],
                                    qks[p0 : p0 + 8, js2],
                                    q=2,
                                )
                else:
                    for jh in range(2):
                        nc.vector.tensor_copy(
                            out=qkb[0:64, jsh[jh]], in_=pph[jh][0:64, :]
                        )
                    for hh in range(2):
                        for qk in range(2):
                            p0 = 32 * hh + 16 * qk
                            dma(
                                qkg_sb[32 * hh : 32 * hh + 16, qk, js2],
                                qkb[p0 : p0 + 16, js2],
                                q=2,
                            )
                            dma(
                                qkg_sb[32 * hh + 16 : 32 * hh + 17, qk, js2],
                                orow_d[0:1, js2],
                                q=2,
                            )

            def vproj(sp_, po_only=False):
                st0 = 2 * sp_
                pv = poP if (po_only or sp_ % 2 == 0) else accP[2]
                for sh in range(2):
                    for k in range(8):
                        nc.tensor.matmul(
                            pv[:, sh * 256 : sh * 256 + 256],
                            lhsT=ht_sb[:, k, ts(st0 + sh, 128)],
                            rhs=wv_sb[:, k, :],
                            start=(k == 0),
                            stop=(k == 7),
                        )
                if po_only or sp_ % 2 == 0:
                    nc.vector.tensor_copy(
                        out=v4r[:, st0 : st0 + 2, :, 0:64],
                        in_=pv[:, 0:512].rearrange(
                            "p (s g x) -> p s g x", s=2, x=64
                        ),
                    )
                else:
                    nc.scalar.copy(
                        out=v4r[:, st0 : st0 + 2, :, 0:64],
                        in_=pv[:, 0:512].rearrange(
                            "p (s g x) -> p s g x", s=2, x=64
                        ),
                    )

            def chunksums():
                for c in range(NT):
                    nc.tensor.matmul(
                        accP[0][0:16, 0:130],
                        lhsT=oneh_sb[:, ts(c, 16)],
                        rhs=v4_sb[:, c, 0:130],
                        start=(c == 0),
                        stop=(c == NT - 1),
                    )
                nc.vector.tensor_copy(out=csum_sb[:, :], in_=accP[0][0:16, 0:130])

            for blk in range(3):
                for jp in range(2):
                    proj_blk(blk, jp)
            for sp_ in range(8):
                vproj(sp_)
            chunksums()
            prefill = []

            # ---------------- phase 2 ----------------
            eng_rr = {"i": 0}

            def rr_copy(out, in_, tail=False):
                # DVE only during phase 2 (keeps the ACT exp+sq queue free of
                # PSUM staging copies); in the drain tail ACT is idle, so
                # alternate to halve the serial copy chain
                eng_rr["i"] += 1
                if tail and eng_rr["i"] % 2 == 0:
                    nc.scalar.copy(out=out, in_=in_)
                else:
                    nc.vector.tensor_copy(out=out, in_=in_)

            fillers = prefill

            def lim_l(j):
                return 4 * j + 4

            def lim_w(j):
                return min(NT, 4 * j + 6)

            def lin_strip(j, ti):
                js0 = j * SB
                col0 = max(0, ti - 4 * j) * 128
                # per-head psum tiles decouple the ACT (head 0) and DVE
                # (head 1) activation chains' WARs on the next strip
                mpa = psA.tile([128, SB], f32, name="mpa", tag="mpa")
                mpb = psA.tile([128, SB], f32, name="mpb", tag="mpb")
                mph = (mpa, mpb)
                if lin_dr:
                    pieces = ([(col0, 256), (256, 512)] if col0 < 256
                              else [(col0, 512)])
                    for h in range(2):
                        for cs, ce in pieces:
                            nc.tensor.matmul(
                                mph[h][:, cs:ce],
                                lhsT=qkg8[32 * h : 32 * h + 8, 1, 0:2, ts(ti, 128)],
                                rhs=qkg8[32 * h : 32 * h + 8, 0, 0:2,
                                         js0 + cs : js0 + ce],
                                start=True,
                                stop=True,
                                perf_mode=DR,
                            )
                else:
                    for h, (p0, p1) in enumerate(((0, 17), (32, 49))):
                        nc.tensor.matmul(
                            mph[h][:, col0:SB],
                            lhsT=qkg_sb[p0:p1, 1, ts(ti, 128)],
                            rhs=qkg_sb[p0:p1, 0, js0 + col0 : js0 + SB],
                            start=True,
                            stop=True,
                        )
                sq = stp.tile([128, 2, SB], bf16, name="sq", tag="st")
                scale = (1.0 / 256.0) if lin_dr else 1.0
                bias = 1.0 if lin_dr else 0.0
                nc.scalar.activation(
                    sq[:, 0, col0:SB], mpa[:, col0:SB], Square,
                    scale=scale, bias=bias,
                )
                mc = mcp.tile([128, SB], bf16, name="mc", tag="mc")
                if lin_dr:
                    nc.vector.tensor_scalar(
                        out=mc[:, col0:SB], in0=mpb[:, col0:SB],
                        scalar1=scale, scalar2=bias,
                        op0=MULT, op1=ADD,
                    )
                else:
                    nc.vector.tensor_copy(
                        out=mc[:, col0:SB], in_=mpb[:, col0:SB]
                    )
                nc.vector.tensor_tensor(
                    out=sq[:, 1, col0:SB], in0=mc[:, col0:SB],
                    in1=mc[:, col0:SB], op=MULT,
                )
                # diagonal causal mask (t <= s)
                sd = ti - 4 * j
                if 0 <= sd <= 3:
                    for h in range(2):
                        nc.gpsimd.tensor_tensor(
                            out=sq[:, h, ts(sd, 128)],
                            in0=sq[:, h, ts(sd, 128)],
                            in1=msk_sb[:, 0:128],
                            op=MULT,
                        )
                return sq

            def win_strip(j, ti):
                js0 = j * SB
                col0 = max(0, ti - 2 - 4 * j) * 128
                sp = psA.tile([128, 1024], f32, name="sp", tag="spab")
                pieces = ([(col0, 256), (256, 512)] if col0 < 256
                          else [(col0, 512)])
                for h in range(2):
                    for cs, ce in pieces:
                        nc.tensor.matmul(
                            sp[:, h * SB + cs : h * SB + ce],
                            lhsT=kw8[32 * h : 32 * h + 32, 0:2, ts(ti, 128)],
                            rhs=qw8[32 * h : 32 * h + 32, 0:2,
                                    js0 + cs : js0 + ce],
                            start=True,
                            stop=True,
                            perf_mode=DR,
                        )
                ex = stp.tile([128, 2, SB], bf16, name="ex", tag="st")
                spr = sp.rearrange("p (g x) -> p g x", x=SB)
                nc.scalar.activation(
                    ex[:, :, col0:SB], spr[:, :, col0:SB], Exp,
                    scale=1.0 / 512.0,
                )
                sd = ti - 2 - 4 * j
                if 0 <= sd <= 3:
                    for h in range(2):
                        nc.gpsimd.tensor_tensor(
                            out=ex[:, h, ts(sd, 128)],
                            in0=ex[:, h, ts(sd, 128)],
                            in1=msk_sb[:, 128:256],
                            op=MULT,
                        )
                return ex

            def acc_mm(strip, j, ti, scl_, g, stop):
                # out[s,65] += strip[:, g%2 head, chunk scl_].T @ v4
                gi = g % 2  # head within strip
                bank_mm(4 * scl_ + g,
                        strip[:, gi, ts(scl_, 128)],
                        v4r[:, ti, g, :], stop)

            def cum_tril(j, scl_, h):
                sc = 4 * j + scl_
                bank_mm(4 * scl_ + h, tril_sb[:, :], v4r[:, sc, h, :], True)

            def cum_base(j, scl_, h):
                sc = 4 * j + scl_
                bank_mm(4 * scl_ + h, bsel_sb[:, ts(sc, 128)],
                        csum_sb[:, 65 * h : 65 * h + 65], False)

            def cum_base0(scl_, h):
                # j=0: sum chunks c < sc via all-ones matmuls (csum not ready)
                for c in range(scl_):
                    bank_mm(4 * scl_ + h, msk_sb[:, 256:384],
                            v4r[:, c, h, :], False)

            def bank_copy(j, b):
                lo = [0, 455, 910][b]
                wdt = [455, 455, 130][b]
                # final block: ACT is idle after the last exp, so split the
                # tail's serial copy chain across both PSUM-capable engines
                if j == NJ - 1 and b == 1:
                    nc.scalar.copy(
                        out=U_sb[:, j, lo : lo + wdt], in_=accP[b][:, 0:wdt]
                    )
                else:
                    nc.vector.tensor_copy(
                        out=U_sb[:, j, lo : lo + wdt], in_=accP[b][:, 0:wdt]
                    )

            def retire_pieces(j, scl_):
                sc = 4 * j + scl_
                pieces = []

                def recip():
                    uj = U_sb.rearrange("p j (G x) -> p j G x", x=65)
                    nc.vector.reciprocal(
                        out=rec_sb[:, j, 4 * scl_ : 4 * scl_ + 4],
                        in_=uj[:, j, 4 * scl_ : 4 * scl_ + 4, 64],
                    )

                pieces.append(recip)

                def div(g):
                    uj = U_sb.rearrange("p j (G x) -> p j G x", x=65)
                    eng = nc.gpsimd
                    eng.tensor_scalar(
                        out=scl_sb[:, sc, 64 * g : 64 * g + 64],
                        in0=uj[:, j, 4 * scl_ + g, 0:64],
                        scalar1=rec_sb[:, j, 4 * scl_ + g : 4 * scl_ + g + 1],
                        scalar2=None,
                        op0=MULT,
                    )

                for g in range(4):
                    pieces.append(lambda g=g: div(g))

                def transp(cg):
                    dma_t(sclT[:, cg, sc, :], scl_sb[:, sc, ts(cg, 128)])

                pieces.append(lambda: transp(0))
                pieces.append(lambda: transp(1))

                so = stg.tile([128, 2, SB], bf16, name="so", tag="so")

                def final(nb):
                    for cg in range(2):
                        nc.tensor.matmul(
                            poP[:, :],
                            lhsT=sclT[:, cg, sc, :],
                            rhs=wo_sb[:, cg, ts(nb, SB)],
                            start=(cg == 0),
                            stop=(cg == 1),
                            skip_group_check=True,
                        )
                    rr_copy(so[:, nb, :], poP[:, :], tail=(j == NJ - 1))

                def store():
                    dma(out_d[ts(sc, 128), :], so.rearrange("p n x -> p (n x)"),
                        q=0)

                pieces.append(lambda: final(0))
                pieces.append(lambda: final(1))
                pieces.append(store)
                return pieces

            def pump(n):
                for _ in range(n):
                    if fillers:
                        fillers.pop(0)()

            LAG = 2

            def emit_accs(j, ti, strips):
                # accumulation for strip ti (strips dict holds live tiles)
                ll, lw = lim_l(j), lim_w(j)
                if ti < ll:
                    sq = strips[("sq", ti)]
                    for scl_ in range(4):
                        sc = 4 * j + scl_
                        if j > 0 and ti == 0 and sc > 0:
                            cum_base(j, scl_, 0)
                            cum_base(j, scl_, 1)
                        if ti < sc:
                            for h in range(2):
                                acc_mm(sq, j, ti, scl_, h, stop=False)
                        elif ti == sc:
                            if j == 0 and scl_ > 0:
                                cum_base0(scl_, 0)
                                cum_base0(scl_, 1)
                            for h in range(2):
                                acc_mm(sq, j, ti, scl_, h, stop=False)
                            cum_tril(j, scl_, 0)
                            cum_tril(j, scl_, 1)
                ex = strips[("ex", ti)]
                for scl_ in range(4):
                    sc = 4 * j + scl_
                    if ti <= min(sc + 2, lw - 1):
                        sp_ = (ti == min(sc + 2, lw - 1))
                        for h in range(2):
                            acc_mm(ex, j, ti, scl_, 2 + h, stop=sp_)

            for j in range(NJ):
                opened.clear()
                ll, lw = lim_l(j), lim_w(j)
                strips = {}
                for ti in range(lw):
                    if ti < ll:
                        strips[("sq", ti)] = lin_strip(j, ti)
                    pump(1)
                    strips[("ex", ti)] = win_strip(j, ti)
                    pump(1)
                    if ti >= LAG:
                        emit_accs(j, ti - LAG, strips)
                    pump(2)
                for ti in range(max(0, lw - LAG), lw):
                    emit_accs(j, ti, strips)
                    pump(1)
                for b in range(3):
                    bank_copy(j, b)
                chunk_pieces = [retire_pieces(j, scl_) for scl_ in range(4)]
                # recip/div/transpose chunk-major, then all finals, then
                # stores: maximizes transpose->final spacing
                for scl_ in range(4):
                    for pi in range(7):
                        fillers.append(chunk_pieces[scl_][pi])
                for pi in (7, 8):
                    for scl_ in range(4):
                        fillers.append(chunk_pieces[scl_][pi])
                for scl_ in range(4):
                    fillers.append(chunk_pieces[scl_][9])
                if j == NJ - 1:
                    pump(len(fillers))
            if dbg:
                dma(dU_d[:, :], U_sb.rearrange("p j x -> p (j x)"), q=0)
                dma(dscl_d[:, :], scl_sb.rearrange("p s x -> p (s x)"), q=0)
                dma(dcs_d[:, :], csum_sb[:, :], q=0)

    nc.compile()
    return nc


def _prep_inputs(inputs):
    h = np.asarray(inputs["hidden_states"], np.float32).reshape(S, D)
    ht = np.ascontiguousarray(h.T).astype(BF)

    lin_Wq = np.asarray(inputs["lin_Wq"], np.float32)
    lin_Wk = np.asarray(inputs["lin_Wk"], np.float32)
    lin_Wv = np.asarray(inputs["lin_Wv"], np.float32)
    lin_Wo = np.asarray(inputs["lin_Wo"], np.float32)
    win_Wq = np.asarray(inputs["win_Wq"], np.float32)
    win_Wk = np.asarray(inputs["win_Wk"], np.float32)
    win_Wv = np.asarray(inputs["win_Wv"], np.float32)
    win_Wo = np.asarray(inputs["win_Wo"], np.float32)

    p = np.arange(128)[:, None]
    f = np.arange(128)[None, :]
    msk = np.zeros((128, 384), np.float32)
    msk[:, 0:128] = (p <= f)
    msk[:, 128:256] = (p < f)
    msk[:, 256:384] = 1.0
    tril = (p <= f).astype(np.float32)        # lhsT[t, s] = t <= s
    bsel = np.zeros((16, NT * 128), np.float32)
    for sc in range(NT):
        bsel[:sc, sc * 128 : (sc + 1) * 128] = 1.0
    oneh = np.zeros((128, NT * 16), np.float32)
    for c in range(NT):
        oneh[:, c * 16 + c] = 1.0

    in_maps = []
    for c in range(NCORES):
        a, b = 2 * c, 2 * c + 1
        wqk = np.zeros((D, 320), np.float32)
        # win q: a_lo a_hi b_lo b_hi (32 each)
        wqk[:, 0:32] = win_Wq[:, a * HD : a * HD + 32]
        wqk[:, 32:64] = win_Wq[:, a * HD + 32 : (a + 1) * HD]
        wqk[:, 64:96] = win_Wq[:, b * HD : b * HD + 32]
        wqk[:, 96:128] = win_Wq[:, b * HD + 32 : (b + 1) * HD]
        wqk[:, 128:160] = win_Wk[:, a * HD : a * HD + 32]
        wqk[:, 160:192] = win_Wk[:, a * HD + 32 : (a + 1) * HD]
        wqk[:, 192:224] = win_Wk[:, b * HD : b * HD + 32]
        wqk[:, 224:256] = win_Wk[:, b * HD + 32 : (b + 1) * HD]
        # lin: qa ka qb kb (16 each), x0.5 fold
        wqk[:, 256:272] = lin_Wq[:, a * FD : (a + 1) * FD] * 0.5
        wqk[:, 272:288] = lin_Wk[:, a * FD : (a + 1) * FD] * 0.5
        wqk[:, 288:304] = lin_Wq[:, b * FD : (b + 1) * FD] * 0.5
        wqk[:, 304:320] = lin_Wk[:, b * FD : (b + 1) * FD] * 0.5
        wv = np.zeros((D, 256), np.float32)
        wv[:, 0:64] = lin_Wv[:, a * HD : (a + 1) * HD] * 0.5
        wv[:, 64:128] = lin_Wv[:, b * HD : (b + 1) * HD] * 0.5
        wv[:, 128:192] = win_Wv[:, a * HD : (a + 1) * HD]
        wv[:, 192:256] = win_Wv[:, b * HD : (b + 1) * HD]
        wo = np.zeros((256, D), np.float32)
        wo[0:64] = lin_Wo[a * HD : (a + 1) * HD]
        wo[64:128] = lin_Wo[b * HD : (b + 1) * HD]
        wo[128:192] = win_Wo[a * HD : (a + 1) * HD]
        wo[192:256] = win_Wo[b * HD : (b + 1) * HD]
        in_maps.append(
            {
                "ht": ht,
                "wqk": wqk.astype(BF),
                "wv": wv.astype(BF),
                "wo": wo.astype(BF),
                "msk": msk.astype(BF),
                "tril": tril.astype(BF),
                "bsel": bsel.astype(BF),
                "oneh": oneh.astype(BF),
                "orow": np.ones((1, S), np.float32).astype(BF),
            }
        )
    return in_maps


def kernel(**inputs) -> np.ndarray:
    from concourse.bass_utils import run_bass_kernel_spmd

    if "nc" not in _CACHE:
        _CACHE["nc"] = _build_nc()
    nc = _CACHE["nc"]
    in_maps = _prep_inputs(inputs)
    res = run_bass_kernel_spmd(nc, in_maps, core_ids=list(range(NCORES)))
    out = np.zeros((S, D), np.float32)
    for r in res.results:
        out += np.asarray(r["out"], np.float32)
    return out.reshape(1, S, D)


if __name__ == "__main__":
    nc = _build_nc()
    print("built ok")

